# revision 1
# baseline (speedup 1.0000x reference)
"""Corr1d-x-group cost-volume kernel for Trainium2 (8 NeuronCores, SPMD).

Data-parallel over batch N=8: core i processes batch i.

Per core (inputs [16, 256, 512] f32 each, output [108, 256, 512] f32):
  out[g*27+ch, h, w] = 0.25 * sum_c f1[g*4+c, h, w] * f2[g*4+c, h, w+ch-23]
with zero padding outside w in [0, 512).

v3 design (vs. baseline):
  - h = hb*8 + hi. Per group g: partitions = (c(4), hb(32)), free = (hi(8), w).
    One fp16 tensor_tensor of FD=4096 per (group, shift) - 108 total - instead
    of 864 FD=512 ops.
  - No zero-padding of f2: shifted slices read junk from neighboring rows /
    guard columns; the wrapped product columns are memset to 0 before the
    channel-reduction matmul (true output there is exactly 0).
  - Odd-parity copy of f2 built on-chip (ScalarE) so odd shifts keep the
    DVE 2x perf mode; r_in is read from HBM only once (~25% less HBM read).
  - Channel reduction on TensorE: weight [128,32] = 0.25 * (c-sum, hb-identity),
    4 groups packed per PSUM bank via tile_position; psum tile [128,1024]
    holds (g,hb) x (hi-pair, w).
  - ScalarE copies PSUM->SBUF staging [128, 4096] f32 per shift; ONE 2 MB
    store per shift (27 total) whose DRAM AP merges to (g, 512KB-contiguous),
    16KB descriptors, alternating between the two HWDGE rings (sync+scalar).
  - A few shifts' multiplies run on GpSimd to offload the DVE.
"""

import os
import numpy as np

import concourse.bass as bass
import concourse.bacc as bacc
import concourse.mybir as mybir
import concourse.tile as tile
from concourse import bass_utils

N, C, H, W = 8, 16, 256, 512
G = 4
TOP_CH = 27
OUT_CH = G * TOP_CH  # 108
HB = 32   # h // 8 -> partition dim component
HI = 8    # h % 8  -> free dim component
FD = HI * W  # 4096
GUARD_L = 24
F2W = GUARD_L + FD + 8  # 4128

_CACHED = {}


def _reduction_weights() -> np.ndarray:
    # lhsT [K=(c,hb)=128, M=hb=32]: sums the 4 channels of a group and
    # applies the 1/sumelems scale.
    w = np.zeros((128, 32), np.float16)
    for c in range(G):
        for hb in range(HB):
            w[c * HB + hb, hb] = 0.25
    return w


def _build_program() -> bass.Bass:
    # Bacc (not raw Bass): its compile() splits multi-sem sync waits, which
    # TRN2 hardware limits to one per instruction.
    nc = bacc.Bacc(
        "TRN2",
        target_bir_lowering=False,
        debug=False,
        enable_asserts=False,
        num_devices=N,
    )
    f16 = mybir.dt.float16
    f32 = mybir.dt.float32

    l_in = nc.dram_tensor("l_in", [C, H, W], f32, kind="ExternalInput")
    r_in = nc.dram_tensor("r_in", [C, H, W], f32, kind="ExternalInput")
    w_red = nc.dram_tensor("w_red", [128, 32], f16, kind="ExternalInput")
    out = nc.dram_tensor("out", [OUT_CH, H, W], f32, kind="ExternalOutput")

    # DRAM views. h = hb*8 + hi.
    l_v = l_in.ap().rearrange(
        "(g c) (hb hi) w -> g (c hb) (hi w)", g=G, hb=HB, hi=HI
    )
    r_v = r_in.ap().rearrange(
        "(g c) (hb hi) w -> g (c hb) (hi w)", g=G, hb=HB, hi=HI
    )
    out_v = out.ap().rearrange(
        "(g s) (hb hi) w -> s g hb (hi w)", g=G, s=TOP_CH, hb=HB, hi=HI
    )

    with tile.TileContext(nc) as tc:
        with (
            tc.tile_pool(name="wpool", bufs=1) as wpool,
            tc.tile_pool(name="inpool", bufs=1) as inpool,
            tc.tile_pool(name="prodpool", bufs=7) as prodpool,
            tc.tile_pool(name="stgpool", bufs=3) as stgpool,
            tc.tile_pool(name="psumpool", bufs=4, space="PSUM") as psumpool,
        ):
            wt = wpool.tile([128, 32], f16)
            nc.sync.dma_start(wt[:], w_red[:])

            f1s, f2es, f2os = [], [], []
            for g in range(G):
                f1 = inpool.tile([128, FD], f16, tag=f"f1_{g}")
                nc.gpsimd.dma_start(f1[:], l_v[g : g + 1])
                f1s.append(f1)
                f2e = inpool.tile([128, F2W], f16, tag=f"f2e_{g}")
                nc.gpsimd.dma_start(
                    f2e[:, GUARD_L : GUARD_L + FD], r_v[g : g + 1]
                )
                f2es.append(f2e)
                # Odd-parity copy: f2o[:, col] = f2e[:, col+1], so odd shifts
                # read 4B-aligned starts (keeps DVE 2x perf mode).
                f2o = inpool.tile([128, F2W], f16, tag=f"f2o_{g}")
                nc.scalar.copy(
                    f2o[:, GUARD_L - 1 : GUARD_L + 2 + FD],
                    f2e[:, GUARD_L : GUARD_L + 3 + FD],
                )
                f2os.append(f2o)

            for s_idx in range(TOP_CH):
                s = s_idx - 23
                stg = stgpool.tile([128, FD], f32, tag="stg")
                # One-time zero of each physical staging buffer: shifted-out
                # columns stay 0 forever (narrowed copies never touch them,
                # and the valid range only expands on later same-buffer
                # shifts, except s>0 whose right edge is re-zeroed below).
                if s_idx < 3:
                    nc.vector.memset(stg[:], 0.0)
                a = max(0, -s)
                b = W - max(0, s)
                stg3 = stg[:].rearrange("x (hi w) -> x hi w", hi=HI)
                ps = []
                for g in range(G):
                    if s % 2 == 0:
                        src = f2es[g][:, GUARD_L + s : GUARD_L + s + FD]
                    else:
                        src = f2os[g][:, GUARD_L - 1 + s : GUARD_L - 1 + s + FD]
                    p = prodpool.tile([128, FD], f16, tag="prod")
                    nc.vector.tensor_mul(p[:], f1s[g][:], src)
                    ps.append(p)
                if s > 0:
                    nc.vector.memset(stg3[:, :, b:W], 0.0)
                for half in range(2):  # hi halves (0-3) / (4-7)
                    for j in range(2 * half, 2 * half + 2):  # hi pairs
                        pt = psumpool.tile([128, 1024], f32, tag="pt")
                        for k in range(2):
                            hi = 2 * j + k
                            for g in range(G):
                                nc.tensor.matmul(
                                    pt[32 * g : 32 * g + 32,
                                       512 * k : 512 * (k + 1)],
                                    wt[:],
                                    ps[g][:, 512 * hi : 512 * (hi + 1)],
                                    start=True,
                                    stop=True,
                                    tile_position=(0, 32 * g),
                                )
                        pt3 = pt[:].rearrange("x (k w) -> x k w", k=2)
                        nc.scalar.copy(
                            stg3[:, 2 * j : 2 * j + 2, a:b], pt3[:, :, a:b]
                        )
                    # Half-size stores spread over 3 DMA rings (SP, SWDGE,
                    # ACT) so descriptor generation overlaps across rings.
                    # The scalar-ring store is issued right after its own
                    # half's copies, so it never blocks later copies.
                    dma_eng = [nc.sync, nc.gpsimd, nc.scalar][
                        (2 * s_idx + half) % 3
                    ]
                    dma_eng.dma_start(
                        out_v[s_idx : s_idx + 1, :, :, 2048 * half : 2048 * (half + 1)],
                        stg[:, 2048 * half : 2048 * (half + 1)],
                    )
    nc.compile()
    return nc


def kernel(l_in: np.ndarray, r_in: np.ndarray) -> np.ndarray:
    assert l_in.shape == (N, C, H, W) and r_in.shape == (N, C, H, W)
    l_in = np.ascontiguousarray(l_in, dtype=np.float32)
    r_in = np.ascontiguousarray(r_in, dtype=np.float32)

    if "nc" not in _CACHED:
        _CACHED["nc"] = _build_program()
    nc = _CACHED["nc"]

    w_np = _reduction_weights()
    in_maps = [
        {
            "l_in": np.ascontiguousarray(l_in[i]),
            "r_in": np.ascontiguousarray(r_in[i]),
            "w_red": w_np,
        }
        for i in range(N)
    ]
    trace = bool(int(os.environ.get("CORR_KERNEL_TRACE", "0")))
    kwargs = {}
    tdir = os.environ.get("CORR_KERNEL_TRACE_DIR")
    if trace and tdir:
        os.makedirs(tdir, exist_ok=True)
        kwargs["tmpdir"] = tdir
    res = bass_utils.run_bass_kernel_spmd(
        nc, in_maps, core_ids=list(range(N)), trace=trace, **kwargs
    )
    _CACHED["last_result"] = res
    return np.stack([res.results[i]["out"] for i in range(N)], axis=0)



# revision 4
# speedup vs baseline: 1.2767x; 1.2767x over previous
"""Corr1d-x-group cost-volume kernel for Trainium2 (8 NeuronCores, SPMD).

Data-parallel over batch N=8: core i processes batch i.

Per core (inputs [16, 256, 512] f32 each, output [108, 256, 512] f32):
  out[g*27+ch, h, w] = 0.25 * sum_c f1[g*4+c, h, w] * f2[g*4+c, h, w+ch-23]
with zero padding outside w in [0, 512).

v4 design (vs. v3 baseline, 514us):
  - Staging buffers are fp16 (not f32); the store DMA casts fp16->f32 in
    the SDMA datapath (SWDGE). Frees 24KB/partition of SBUF -> prodpool
    grows 7->9 bufs so the DVE never stalls on product-buffer reuse.
  - One 2MB store per shift on the SWDGE ring (no ring rotation needed).
  - On alternate shifts one of the four group-products runs on GpSimd,
    trimming DVE busy from ~247us to ~210us.
  - ScalarE does only the PSUM->SBUF(fp16) copies + the odd-parity f2
    copy; memsets stay off the critical DVE path where possible.
"""

import os
import numpy as np

import concourse.bass as bass
import concourse.bacc as bacc
import concourse.mybir as mybir
import concourse.tile as tile
from concourse import bass_utils

N, C, H, W = 8, 16, 256, 512
G = 4
TOP_CH = 27
OUT_CH = G * TOP_CH  # 108
HB = 32   # h // 8 -> partition dim component
HI = 8    # h % 8  -> free dim component
FD = HI * W  # 4096
GUARD_L = 24
F2W = GUARD_L + FD + 8  # 4128

STG_BUFS = 4
PROD_BUFS = 9
# Shifts whose g=3 product runs on GpSimd instead of DVE.
GPSIMD_SHIFTS = frozenset(range(2, 27, 2))

_CACHED = {}


def _reduction_weights() -> np.ndarray:
    # lhsT [K=(c,hb)=128, M=hb=32]: sums the 4 channels of a group and
    # applies the 1/sumelems scale.
    w = np.zeros((128, 32), np.float16)
    for c in range(G):
        for hb in range(HB):
            w[c * HB + hb, hb] = 0.25
    return w


def _build_program() -> bass.Bass:
    # Bacc (not raw Bass): its compile() splits multi-sem sync waits, which
    # TRN2 hardware limits to one per instruction.
    nc = bacc.Bacc(
        "TRN2",
        target_bir_lowering=False,
        debug=False,
        enable_asserts=False,
        num_devices=N,
    )
    f16 = mybir.dt.float16
    f32 = mybir.dt.float32

    l_in = nc.dram_tensor("l_in", [C, H, W], f32, kind="ExternalInput")
    r_in = nc.dram_tensor("r_in", [C, H, W], f32, kind="ExternalInput")
    w_red = nc.dram_tensor("w_red", [128, 32], f16, kind="ExternalInput")
    out = nc.dram_tensor("out", [OUT_CH, H, W], f32, kind="ExternalOutput")

    # DRAM views. h = hb*8 + hi.
    l_v = l_in.ap().rearrange(
        "(g c) (hb hi) w -> g (c hb) (hi w)", g=G, hb=HB, hi=HI
    )
    r_v = r_in.ap().rearrange(
        "(g c) (hb hi) w -> g (c hb) (hi w)", g=G, hb=HB, hi=HI
    )
    out_v = out.ap().rearrange(
        "(g s) (hb hi) w -> s g hb (hi w)", g=G, s=TOP_CH, hb=HB, hi=HI
    )

    with tile.TileContext(nc) as tc:
        with (
            tc.tile_pool(name="wpool", bufs=1) as wpool,
            tc.tile_pool(name="inpool", bufs=1) as inpool,
            tc.tile_pool(name="prodpool", bufs=PROD_BUFS) as prodpool,
            tc.tile_pool(name="stgpool", bufs=STG_BUFS) as stgpool,
            tc.tile_pool(name="psumpool", bufs=4, space="PSUM") as psumpool,
        ):
            wt = wpool.tile([128, 32], f16)
            nc.sync.dma_start(wt[:], w_red[:])

            f1s, f2es, f2os = [], [], []
            for g in range(G):
                f1 = inpool.tile([128, FD], f16, tag=f"f1_{g}")
                nc.gpsimd.dma_start(f1[:], l_v[g : g + 1])
                f1s.append(f1)
                f2e = inpool.tile([128, F2W], f16, tag=f"f2e_{g}")
                nc.gpsimd.dma_start(
                    f2e[:, GUARD_L : GUARD_L + FD], r_v[g : g + 1]
                )
                f2es.append(f2e)
                # Odd-parity copy: f2o[:, col] = f2e[:, col+1], so odd shifts
                # read 4B-aligned starts (keeps DVE 2x perf mode).
                f2o = inpool.tile([128, F2W], f16, tag=f"f2o_{g}")
                nc.scalar.copy(
                    f2o[:, GUARD_L - 1 : GUARD_L + 2 + FD],
                    f2e[:, GUARD_L : GUARD_L + 3 + FD],
                )
                f2os.append(f2o)

            for s_idx in range(TOP_CH):
                s = s_idx - 23
                stg = stgpool.tile([128, FD], f16, tag="stg")
                # One-time zero of each physical staging buffer: shifted-out
                # columns stay 0 forever (narrowed copies never touch them,
                # and the valid range only expands on later same-buffer
                # shifts, except s>0 whose right edge is re-zeroed below).
                if s_idx < STG_BUFS:
                    nc.vector.memset(stg[:], 0.0)
                a = max(0, -s)
                b = W - max(0, s)
                stg3 = stg[:].rearrange("x (hi w) -> x hi w", hi=HI)
                ps = []
                for g in range(G):
                    if s % 2 == 0:
                        src = f2es[g][:, GUARD_L + s : GUARD_L + s + FD]
                    else:
                        src = f2os[g][:, GUARD_L - 1 + s : GUARD_L - 1 + s + FD]
                    p = prodpool.tile([128, FD], f16, tag="prod")
                    eng = (
                        nc.gpsimd
                        if (g == 3 and s_idx in GPSIMD_SHIFTS)
                        else nc.vector
                    )
                    eng.tensor_mul(p[:], f1s[g][:], src)
                    ps.append(p)
                if s > 0:
                    nc.vector.memset(stg3[:, :, b:W], 0.0)
                for j in range(4):  # hi pairs
                    pt = psumpool.tile([128, 1024], f32, tag="pt")
                    for k in range(2):
                        hi = 2 * j + k
                        for g in range(G):
                            nc.tensor.matmul(
                                pt[32 * g : 32 * g + 32,
                                   512 * k : 512 * (k + 1)],
                                wt[:],
                                ps[g][:, 512 * hi : 512 * (hi + 1)],
                                start=True,
                                stop=True,
                                tile_position=(0, 32 * g),
                            )
                    pt3 = pt[:].rearrange("x (k w) -> x k w", k=2)
                    nc.scalar.copy(
                        stg3[:, 2 * j : 2 * j + 2, a:b], pt3[:, :, a:b]
                    )
                # Single whole-shift store; SWDGE casts fp16 -> f32 inline.
                nc.gpsimd.dma_start(out_v[s_idx : s_idx + 1], stg[:])
    nc.compile()
    return nc


def kernel(l_in: np.ndarray, r_in: np.ndarray) -> np.ndarray:
    assert l_in.shape == (N, C, H, W) and r_in.shape == (N, C, H, W)
    l_in = np.ascontiguousarray(l_in, dtype=np.float32)
    r_in = np.ascontiguousarray(r_in, dtype=np.float32)

    if "nc" not in _CACHED:
        _CACHED["nc"] = _build_program()
    nc = _CACHED["nc"]

    w_np = _reduction_weights()
    in_maps = [
        {
            "l_in": np.ascontiguousarray(l_in[i]),
            "r_in": np.ascontiguousarray(r_in[i]),
            "w_red": w_np,
        }
        for i in range(N)
    ]
    trace = bool(int(os.environ.get("CORR_KERNEL_TRACE", "0")))
    kwargs = {}
    tdir = os.environ.get("CORR_KERNEL_TRACE_DIR")
    if trace and tdir:
        os.makedirs(tdir, exist_ok=True)
        kwargs["tmpdir"] = tdir
    res = bass_utils.run_bass_kernel_spmd(
        nc, in_maps, core_ids=list(range(N)), trace=trace, **kwargs
    )
    _CACHED["last_result"] = res
    return np.stack([res.results[i]["out"] for i in range(N)], axis=0)


# revision 8
# speedup vs baseline: 1.5882x; 1.2440x over previous
"""Corr1d-x-group cost-volume kernel for Trainium2 (8 NeuronCores, SPMD).

Data-parallel over batch N=8: core i processes batch i.

Per core (inputs [16, 256, 512] f32 each, output [108, 256, 512] f32):
  out[g*27+ch, h, w] = 0.25 * sum_c f1[g*4+c, h, w] * f2[g*4+c, h, w+ch-23]
with zero padding outside w in [0, 512).

v5 design (vs. v3 baseline, 514us):
  - Staging buffers are fp16 (not f32); the store DMA casts fp16->f32 in
    the SDMA datapath (SWDGE). Frees 24KB/partition of SBUF -> prodpool
    grows 7->9 bufs so the DVE never stalls on product-buffer reuse.
  - One 2MB store per shift on the SWDGE ring (no ring rotation needed).
  - All 108 multiplies on DVE (GpSimd tensor_tensor was tried and hurts:
    concurrent Q7 SBUF traffic inflates DVE TT from 2.28us to 4.1us).
  - Staging-buffer zeroing on GpSimd before the loop (off the DVE path);
    first processed shift is even so warmup skips the f2o dependency.
  - ScalarE does only the PSUM->SBUF(fp16) copies + the odd-parity f2
    copy.
"""

import os
import numpy as np

import concourse.bass as bass
import concourse.bacc as bacc
import concourse.mybir as mybir
import concourse.tile as tile
from concourse import bass_utils

N, C, H, W = 8, 16, 256, 512
G = 4
TOP_CH = 27
OUT_CH = G * TOP_CH  # 108
HB = 32   # h // 8 -> partition dim component
HI = 8    # h % 8  -> free dim component
FD = HI * W  # 4096
GUARD_L = 24
F2W = GUARD_L + FD + 8  # 4128

STG_BUFS = 4
PROD_BUFS = 9
# First processed shift is even (s=-22): no dependency on the odd-parity
# copy, so the pipeline starts as soon as f1/f2e loads land.
SHIFT_ORDER = [1, 0] + list(range(2, TOP_CH))

_CACHED = {}


def _reduction_weights() -> np.ndarray:
    # lhsT [K=(c,hb)=128, M=hb=32]: sums the 4 channels of a group and
    # applies the 1/sumelems scale.
    w = np.zeros((128, 32), np.float16)
    for c in range(G):
        for hb in range(HB):
            w[c * HB + hb, hb] = 0.25
    return w


def _build_program() -> bass.Bass:
    # Bacc (not raw Bass): its compile() splits multi-sem sync waits, which
    # TRN2 hardware limits to one per instruction.
    nc = bacc.Bacc(
        "TRN2",
        target_bir_lowering=False,
        debug=False,
        enable_asserts=False,
        num_devices=N,
    )
    f16 = mybir.dt.float16
    f32 = mybir.dt.float32

    l_in = nc.dram_tensor("l_in", [C, H, W], f32, kind="ExternalInput")
    r_in = nc.dram_tensor("r_in", [C, H, W], f32, kind="ExternalInput")
    w_red = nc.dram_tensor("w_red", [128, 32], f16, kind="ExternalInput")
    out = nc.dram_tensor("out", [OUT_CH, H, W], f32, kind="ExternalOutput")

    # DRAM views. h = hb*8 + hi.
    l_v = l_in.ap().rearrange(
        "(g c) (hb hi) w -> g (c hb) (hi w)", g=G, hb=HB, hi=HI
    )
    r_v = r_in.ap().rearrange(
        "(g c) (hb hi) w -> g (c hb) (hi w)", g=G, hb=HB, hi=HI
    )
    out_v = out.ap().rearrange(
        "(g s) (hb hi) w -> s g hb (hi w)", g=G, s=TOP_CH, hb=HB, hi=HI
    )

    with tile.TileContext(nc) as tc:
        with (
            tc.tile_pool(name="wpool", bufs=1) as wpool,
            tc.tile_pool(name="inpool", bufs=1) as inpool,
            tc.tile_pool(name="prodpool", bufs=PROD_BUFS) as prodpool,
            tc.tile_pool(name="stgpool", bufs=STG_BUFS) as stgpool,
            tc.tile_pool(name="psumpool", bufs=4, space="PSUM") as psumpool,
        ):
            wt = wpool.tile([128, 32], f16)
            nc.sync.dma_start(wt[:], w_red[:])

            f1s, f2es, f2os = [], [], []
            for g in range(G):
                f1 = inpool.tile([128, FD], f16, tag=f"f1_{g}")
                nc.gpsimd.dma_start(f1[:], l_v[g : g + 1])
                f1s.append(f1)
                f2e = inpool.tile([128, F2W], f16, tag=f"f2e_{g}")
                nc.gpsimd.dma_start(
                    f2e[:, GUARD_L : GUARD_L + FD], r_v[g : g + 1]
                )
                f2es.append(f2e)
                # Odd-parity copy: f2o[:, col] = f2e[:, col+1], so odd shifts
                # read 4B-aligned starts (keeps DVE 2x perf mode).
                f2o = inpool.tile([128, F2W], f16, tag=f"f2o_{g}")
                nc.scalar.copy(
                    f2o[:, GUARD_L - 1 : GUARD_L + 2 + FD],
                    f2e[:, GUARD_L : GUARD_L + 3 + FD],
                )
                f2os.append(f2o)

            # One-time zero of each physical staging buffer (GpSimd: runs
            # during the load phase while the DVE is idle): shifted-out
            # columns stay 0 forever (narrowed copies never touch them, and
            # the valid range only expands on later same-buffer shifts,
            # except s>0 whose right edge is re-zeroed below).
            for _ in range(STG_BUFS):
                stg = stgpool.tile([128, FD], f16, tag="stg")
                nc.gpsimd.memset(stg[:], 0.0)

            for s_idx in SHIFT_ORDER:
                s = s_idx - 23
                stg = stgpool.tile([128, FD], f16, tag="stg")
                a = max(0, -s)
                b = W - max(0, s)
                stg3 = stg[:].rearrange("x (hi w) -> x hi w", hi=HI)
                ps = []
                for g in range(G):
                    if s % 2 == 0:
                        src = f2es[g][:, GUARD_L + s : GUARD_L + s + FD]
                    else:
                        src = f2os[g][:, GUARD_L - 1 + s : GUARD_L - 1 + s + FD]
                    p = prodpool.tile([128, FD], f16, tag="prod")
                    nc.vector.tensor_mul(p[:], f1s[g][:], src)
                    ps.append(p)
                if s > 0:
                    nc.vector.memset(stg3[:, :, b:W], 0.0)
                for j in range(4):  # hi pairs
                    pt = psumpool.tile([128, 1024], f32, tag="pt")
                    for k in range(2):
                        hi = 2 * j + k
                        for g in range(G):
                            nc.tensor.matmul(
                                pt[32 * g : 32 * g + 32,
                                   512 * k : 512 * (k + 1)],
                                wt[:],
                                ps[g][:, 512 * hi : 512 * (hi + 1)],
                                start=True,
                                stop=True,
                                tile_position=(0, 32 * g),
                            )
                    pt3 = pt[:].rearrange("x (k w) -> x k w", k=2)
                    nc.scalar.copy(
                        stg3[:, 2 * j : 2 * j + 2, a:b], pt3[:, :, a:b]
                    )
                # Single whole-shift store; SWDGE casts fp16 -> f32 inline.
                nc.gpsimd.dma_start(out_v[s_idx : s_idx + 1], stg[:])
    nc.compile()
    return nc


def kernel(l_in: np.ndarray, r_in: np.ndarray) -> np.ndarray:
    assert l_in.shape == (N, C, H, W) and r_in.shape == (N, C, H, W)
    l_in = np.ascontiguousarray(l_in, dtype=np.float32)
    r_in = np.ascontiguousarray(r_in, dtype=np.float32)

    if "nc" not in _CACHED:
        _CACHED["nc"] = _build_program()
    nc = _CACHED["nc"]

    w_np = _reduction_weights()
    in_maps = [
        {
            "l_in": np.ascontiguousarray(l_in[i]),
            "r_in": np.ascontiguousarray(r_in[i]),
            "w_red": w_np,
        }
        for i in range(N)
    ]
    trace = bool(int(os.environ.get("CORR_KERNEL_TRACE", "0")))
    kwargs = {}
    tdir = os.environ.get("CORR_KERNEL_TRACE_DIR")
    if trace and tdir:
        os.makedirs(tdir, exist_ok=True)
        kwargs["tmpdir"] = tdir
    res = bass_utils.run_bass_kernel_spmd(
        nc, in_maps, core_ids=list(range(N)), trace=trace, **kwargs
    )
    _CACHED["last_result"] = res
    return np.stack([res.results[i]["out"] for i in range(N)], axis=0)


# revision 12
# speedup vs baseline: 1.6799x; 1.0577x over previous
"""Corr1d-x-group cost-volume kernel for Trainium2 (8 NeuronCores, SPMD).

Data-parallel over batch N=8: core i processes batch i.

Per core (inputs [16, 256, 512] f32 each, output [108, 256, 512] f32):
  out[g*27+ch, h, w] = 0.25 * sum_c f1[g*4+c, h, w] * f2[g*4+c, h, w+ch-23]
with zero padding outside w in [0, 512).

v6 design (vs. v3 baseline, 514us; v5, 324us):
  - f2 is stored with 24 zero columns between the 8 hi-segments
    (segment stride 536). Shifted product reads then pull exact zeros
    outside the valid w-range instead of neighbor-row junk, so products
    are correct everywhere: no staging-buffer zeroing, no narrowed
    PSUM->SBUF copies, no per-shift edge memsets.
  - Staging buffers are fp16; the store DMA casts fp16->f32 inline
    (SWDGE). Frees SBUF -> prodpool 8 bufs, stgpool 4.
  - Stores in 1MB halves per shift (earlier release, smoother SDMA).
  - All 108 multiplies on DVE (GpSimd tensor_tensor contends with DVE
    for SBUF ports: tried and reverted).
  - ScalarE does only PSUM->SBUF(fp16) copies + the odd-parity f2 copy.
"""

import os
import numpy as np

import concourse.bass as bass
import concourse.bacc as bacc
import concourse.mybir as mybir
import concourse.tile as tile
from concourse import bass_utils

N, C, H, W = 8, 16, 256, 512
G = 4
TOP_CH = 27
OUT_CH = G * TOP_CH  # 108
HB = 32   # h // 8 -> partition dim component
HI = 8    # h % 8  -> free dim component
FD = HI * W  # 4096
GAP = 24          # zero columns before each hi segment (covers s in [-23,3])
SEG = GAP + W     # 536
# 8 segments + trailing zeros; sized so the widest shifted [HI*SEG] slice
# (start GAP+3 even / GAP-1+3 odd) stays in bounds: >= GAP+3+HI*SEG = 4315.
F2W = HI * SEG + 32  # 4320

STG_BUFS = 4
PROD_BUFS = 8
# First processed shift is even (s=-22): no dependency on the odd-parity
# copy, so the pipeline starts as soon as f1/f2e loads land.
SHIFT_ORDER = [1, 0] + list(range(2, TOP_CH))

_CACHED = {}


def _reduction_weights() -> np.ndarray:
    # lhsT [K=(c,hb)=128, M=hb=32]: sums the 4 channels of a group and
    # applies the 1/sumelems scale.
    w = np.zeros((128, 32), np.float16)
    for c in range(G):
        for hb in range(HB):
            w[c * HB + hb, hb] = 0.25
    return w


def _build_program() -> bass.Bass:
    # Bacc (not raw Bass): its compile() splits multi-sem sync waits, which
    # TRN2 hardware limits to one per instruction.
    nc = bacc.Bacc(
        "TRN2",
        target_bir_lowering=False,
        debug=False,
        enable_asserts=False,
        num_devices=N,
    )
    f16 = mybir.dt.float16
    f32 = mybir.dt.float32

    l_in = nc.dram_tensor("l_in", [C, H, W], f32, kind="ExternalInput")
    r_in = nc.dram_tensor("r_in", [C, H, W], f32, kind="ExternalInput")
    w_red = nc.dram_tensor("w_red", [128, 32], f16, kind="ExternalInput")
    out = nc.dram_tensor("out", [OUT_CH, H, W], f32, kind="ExternalOutput")

    # DRAM views. h = hb*8 + hi.
    l_v = l_in.ap().rearrange(
        "(g c) (hb hi) w -> g (c hb) (hi w)", g=G, hb=HB, hi=HI
    )
    r_v = r_in.ap().rearrange(
        "(g c) (hb hi) w -> g (c hb) hi w", g=G, hb=HB, hi=HI
    )
    out_v = out.ap().rearrange(
        "(g s) (hb hi) w -> s g hb (hi w)", g=G, s=TOP_CH, hb=HB, hi=HI
    )

    with tile.TileContext(nc) as tc:
        with (
            tc.tile_pool(name="wpool", bufs=1) as wpool,
            tc.tile_pool(name="inpool", bufs=1) as inpool,
            tc.tile_pool(name="prodpool", bufs=PROD_BUFS) as prodpool,
            tc.tile_pool(name="stgpool", bufs=STG_BUFS) as stgpool,
            tc.tile_pool(name="psumpool", bufs=4, space="PSUM") as psumpool,
        ):
            wt = wpool.tile([128, 32], f16)
            nc.sync.dma_start(wt[:], w_red[:])

            f1s, f2es, f2os = [], [], []
            for g in range(G):
                f1 = inpool.tile([128, FD], f16, tag=f"f1_{g}")
                nc.gpsimd.dma_start(f1[:], l_v[g : g + 1])
                f1s.append(f1)
                f2e = inpool.tile([128, F2W], f16, tag=f"f2e_{g}")
                # Zero the inter-segment gaps + trailing columns (tiny DVE
                # memsets; run during the load phase while the DVE is idle).
                f2e_seg = f2e[:, : HI * SEG].rearrange(
                    "x (hi c) -> x hi c", hi=HI
                )
                nc.vector.memset(f2e_seg[:, :, 0:GAP], 0.0)
                nc.vector.memset(f2e[:, HI * SEG :], 0.0)
                nc.gpsimd.dma_start(f2e_seg[:, :, GAP:SEG], r_v[g])
                f2es.append(f2e)
                # Odd-parity copy: f2o[:, col] = f2e[:, col+1], so odd shifts
                # read 4B-aligned starts (keeps DVE 2x perf mode).
                f2o = inpool.tile([128, F2W], f16, tag=f"f2o_{g}")
                nc.scalar.copy(f2o[:, : F2W - 2], f2e[:, 1 : F2W - 1])
                nc.vector.memset(f2o[:, F2W - 2 :], 0.0)
                f2os.append(f2o)

            for s_idx in SHIFT_ORDER:
                s = s_idx - 23
                stg = stgpool.tile([128, FD], f16, tag="stg")
                ps = []
                for g in range(G):
                    if s % 2 == 0:
                        src = f2es[g][:, GAP + s : GAP + s + HI * SEG]
                    else:
                        src = f2os[g][:, GAP - 1 + s : GAP - 1 + s + HI * SEG]
                    src3 = src.rearrange("x (hi c) -> x hi c", hi=HI)
                    p = prodpool.tile([128, FD], f16, tag="prod")
                    p3 = p[:].rearrange("x (hi w) -> x hi w", hi=HI)
                    nc.vector.tensor_mul(p3[:], _f13(f1s[g]), src3[:, :, 0:W])
                    ps.append(p)
                for j in range(4):  # hi pairs
                    pt = psumpool.tile([128, 1024], f32, tag="pt")
                    for k in range(2):
                        hi = 2 * j + k
                        for g in range(G):
                            nc.tensor.matmul(
                                pt[32 * g : 32 * g + 32,
                                   512 * k : 512 * (k + 1)],
                                wt[:],
                                ps[g][:, 512 * hi : 512 * (hi + 1)],
                                start=True,
                                stop=True,
                                tile_position=(0, 32 * g),
                            )
                    nc.scalar.copy(stg[:, 1024 * j : 1024 * (j + 1)], pt[:])
                    if j % 2 == 1:
                        # Store the finished half (SWDGE casts fp16 -> f32).
                        half = j // 2
                        nc.gpsimd.dma_start(
                            out_v[
                                s_idx : s_idx + 1, :, :,
                                2048 * half : 2048 * (half + 1),
                            ],
                            stg[:, 2048 * half : 2048 * (half + 1)],
                        )
    nc.compile()
    return nc


def _f13(f1):
    return f1[:].rearrange("x (hi w) -> x hi w", hi=HI)


def kernel(l_in: np.ndarray, r_in: np.ndarray) -> np.ndarray:
    assert l_in.shape == (N, C, H, W) and r_in.shape == (N, C, H, W)
    l_in = np.ascontiguousarray(l_in, dtype=np.float32)
    r_in = np.ascontiguousarray(r_in, dtype=np.float32)

    if "nc" not in _CACHED:
        _CACHED["nc"] = _build_program()
    nc = _CACHED["nc"]

    w_np = _reduction_weights()
    in_maps = [
        {
            "l_in": np.ascontiguousarray(l_in[i]),
            "r_in": np.ascontiguousarray(r_in[i]),
            "w_red": w_np,
        }
        for i in range(N)
    ]
    trace = bool(int(os.environ.get("CORR_KERNEL_TRACE", "0")))
    kwargs = {}
    tdir = os.environ.get("CORR_KERNEL_TRACE_DIR")
    if trace and tdir:
        os.makedirs(tdir, exist_ok=True)
        kwargs["tmpdir"] = tdir
    res = bass_utils.run_bass_kernel_spmd(
        nc, in_maps, core_ids=list(range(N)), trace=trace, **kwargs
    )
    _CACHED["last_result"] = res
    return np.stack([res.results[i]["out"] for i in range(N)], axis=0)
